# revision 2
# baseline (speedup 1.0000x reference)
"""Trainium2 Bass kernel for nn_DiffuseLR (SGConv K=1 diffusion + FC + softmax).

Math: out = softmax(x @ A^T @ fc_w^T + fc_b) with A the gcn-normalized
adjacency (incl. self loops). We exploit linearity: S^T = A^T @ fc_w^T is a
(N x C) "diffused classifier" computed by gather + one-hot matmul
(segment-sum over src), then logits = x @ S^T. The N-dim contraction is
sharded over 8 cores by src-node groups; per-core partial logits (128 x 30)
are AllReduced on-device and every core computes the final softmax.

Per-core device work:
  - dma_gather fc_w^T rows by edge dst (256B rows) for its edge shard
  - one-hot (src-relative, wn-weighted) matmuls accumulate S^T tiles in PSUM
  - FC phase: x^T tiles (host-permuted to this core's node order) @ S^T
  - AllReduce partial logits, add bias, softmax

Host side only: gcn normalization (per the sharding hint, the normalized
edge weight is replicated data), sorting/sharding of edges, and data layout
(transposes / padding / index packing).
"""
import os
import numpy as np

from concourse import bacc, bass, mybir, tile
from concourse.bass_utils import run_bass_kernel_spmd

N_CORES = 8
N = 20000          # nodes (features of x)
B = 128            # batch
C = 30             # classes
P = 128            # edges per matmul chunk (contraction dim)
W = 32             # src-node group width (onehot width)
BLK = 32           # chunks per gather/onehot block
ELEM = 64          # gathered row width in f32 (256 B, fc row padded 30->64)
GB = 8             # chunks per dma_gather instruction (1024 idx, 64
                   # descriptors per SDMA engine = single-packet limit)
NQ = 4             # SWDGE queues; queue q runs on Q7 cores 2q,2q+1 so 4
                   # queues parallelize descriptor generation 4x (measured)
G = N // W         # 625 src groups
F32 = mybir.dt.float32
I16 = mybir.dt.int16

_CACHE = {}


# ---------------------------------------------------------------------------
# workaround: this neuronxcc build accepts at most ONE sync wait per
# instruction; move extra waits onto same-engine NoOps placed just before.
def _split_multi_waits(nc):
    n_split = 0
    for func in nc.m.functions:
        for bb in func.blocks:
            insts = list(bb.instructions)
            if not any(
                i.sync_info is not None and len(i.sync_info.on_wait) > 1
                for i in insts
            ):
                continue
            new_list = []
            for inst in insts:
                si = inst.sync_info
                if si is not None and len(si.on_wait) > 1:
                    waits = list(si.on_wait)
                    for w in waits[:-1]:
                        nop = nc.engines[inst.engine].nop(
                            nofuse=True, hint="waitsplit"
                        )
                        cur = nc.cur_bb.bb
                        assert cur.instructions[-1].name == nop.ins.name
                        cur.instructions.pop()
                        nop.ins.sync_info = mybir.SyncInfo(
                            on_wait=[w], on_update=[]
                        )
                        new_list.append(nop.ins)
                        n_split += 1
                    si.on_wait = waits[-1:]
                new_list.append(inst)
            bb.instructions.clear()
            for i in new_list:
                bb.instructions.append(i)
    return n_split


# ---------------------------------------------------------------------------
def _preprocess(x, edge_index, edge_weight, fc_w, fc_b):
    """Host: normalize, sort by src, shard groups to cores, pack arrays."""
    src = np.asarray(edge_index[0], dtype=np.int64)
    dst = np.asarray(edge_index[1], dtype=np.int64)
    w = np.asarray(edge_weight, dtype=np.float64)
    loop = np.arange(N, dtype=np.int64)
    src_f = np.concatenate([src, loop])
    dst_f = np.concatenate([dst, loop])
    w_f = np.concatenate([w, np.ones(N)])

    deg = np.bincount(dst_f, weights=w_f, minlength=N)
    dinv = np.where(deg > 0, 1.0 / np.sqrt(np.maximum(deg, 1e-30)), 0.0)
    # self-loops are NOT gathered: their contribution is the diagonal term
    # S^T[n,:] += dinv[n]^2 * fcT[n,:], applied on-device after phase 1.
    wn = (dinv[src] * w * dinv[dst]).astype(np.float32)
    dinv2 = (dinv * dinv).astype(np.float32)

    order = np.argsort(src, kind="stable")
    src_s = src[order]
    dst_s = dst[order].astype(np.int16)
    wn_s = wn[order]

    gid = (src_s // W).astype(np.int64)
    cnt = np.bincount(gid, minlength=G)
    gstart = np.zeros(G + 1, np.int64)
    np.cumsum(cnt, out=gstart[1:])
    srel_all = (src_s - gid * W).astype(np.float32)

    chk = np.maximum((cnt + P - 1) // P, 1)
    meta_nch = None  # set below
    gorder = np.argsort(-cnt, kind="stable")
    nslot = (G + N_CORES - 1) // N_CORES               # 79
    K = np.array([chk[gorder[s * N_CORES]] for s in range(nslot)], np.int64)
    nch = int(K.sum())
    meta_nch = nch
    nchp = ((nch + BLK - 1) // BLK) * BLK
    nblk = nchp // BLK
    chunk_off = np.zeros(nslot + 1, np.int64)
    np.cumsum(K, out=chunk_off[1:])

    xT = np.ascontiguousarray(np.asarray(x, np.float32).T)  # (N, B)
    ntile = (nslot * W + 127) // 128                        # 20
    T = nchp * P

    table = np.zeros((N, ELEM), np.float32)
    table[:, :C] = np.asarray(fc_w, np.float32).T

    iota = np.broadcast_to(np.arange(W, dtype=np.float32), (128, W)).copy()
    bias = np.broadcast_to(np.asarray(fc_b, np.float32), (128, C)).copy()

    in_maps = []
    for c in range(N_CORES):
        gdst = np.zeros(T, np.int16)
        srel = np.full(T, 99.0, np.float32)
        wnv = np.zeros(T, np.float32)
        node_ids = np.zeros(ntile * 128, np.int64)
        valid_row = np.zeros(ntile * 128, bool)
        for s in range(nslot):
            r = s * N_CORES + c
            if r >= G:
                continue
            g = gorder[r]
            e0, n_e = gstart[g], cnt[g]
            base = chunk_off[s] * P
            gdst[base:base + n_e] = dst_s[e0:e0 + n_e]
            srel[base:base + n_e] = srel_all[e0:e0 + n_e]
            wnv[base:base + n_e] = wn_s[e0:e0 + n_e]
            node_ids[s * W:(s + 1) * W] = g * W + np.arange(W)
            valid_row[s * W:(s + 1) * W] = True
        # trailing dead chunks: -1 indices are trimmed by the gather ucode
        nch_edges = int(meta_nch) * P
        gdst[nch_edges:] = -1
        # idx wrapped per gather block: within block, flat k -> [k%16, k//16]
        idxw = np.concatenate(
            [gdst[b * BLK * P:(b + 1) * BLK * P].reshape(-1, 16).T
             for b in range(nblk)], axis=1)           # (16, nblk*BLK*8)
        idxw = np.tile(idxw, (8, 1))                  # (128, ...)
        in_maps.append({
            "table": table,
            "idxw": np.ascontiguousarray(idxw),
            "srel": np.ascontiguousarray(srel.reshape(nchp, P).T),
            "wnv": np.ascontiguousarray(wnv.reshape(nchp, P).T),
            "xTp": np.ascontiguousarray(xT[node_ids, :]),
            "fcp": np.ascontiguousarray(table[node_ids, :W]),
            "d2": np.ascontiguousarray(
                (dinv2[node_ids] * valid_row).astype(np.float32)
                .reshape(ntile, 128).T),
            "iota": iota,
            "bias": bias,
        })
    meta = {"nslot": nslot, "K": tuple(int(k) for k in K),
            "nchp": nchp, "nblk": nblk, "ntile": ntile, "nch": nch}
    return in_maps, meta


# ---------------------------------------------------------------------------
def _build(meta, phases=4, max_slots=None, repeat=1):
    nslot, K, nchp, nblk, ntile = (meta["nslot"], meta["K"], meta["nchp"],
                                   meta["nblk"], meta["ntile"])
    nch_edges = meta["nch"] * P
    if max_slots is not None:
        nslot = min(nslot, max_slots)
    nc = bacc.Bacc("TRN2", debug=False, enable_asserts=False,
                   num_devices=N_CORES, num_swdge_queues=NQ)
    t_table = nc.dram_tensor("table", [N, ELEM], F32, kind="ExternalInput").ap()
    t_idxw = nc.dram_tensor("idxw", [128, nblk * BLK * 8], I16,
                            kind="ExternalInput").ap()
    t_srel = nc.dram_tensor("srel", [128, nchp], F32, kind="ExternalInput").ap()
    t_wnv = nc.dram_tensor("wnv", [128, nchp], F32, kind="ExternalInput").ap()
    t_xTp = nc.dram_tensor("xTp", [ntile * 128, B], F32,
                           kind="ExternalInput").ap()
    t_fcp = nc.dram_tensor("fcp", [ntile * 128, W], F32,
                           kind="ExternalInput").ap()
    t_d2 = nc.dram_tensor("d2", [128, ntile], F32, kind="ExternalInput").ap()
    t_iota = nc.dram_tensor("iota", [128, W], F32, kind="ExternalInput").ap()
    t_bias = nc.dram_tensor("bias", [128, C], F32, kind="ExternalInput").ap()
    t_out = nc.dram_tensor("out", [B, C], F32, kind="ExternalOutput").ap()

    with tile.TileContext(nc) as tc:
        with tc.tile_pool(name="const", bufs=1) as const, \
             tc.tile_pool(name="gath", bufs=int(os.environ.get("KBUFS", "4"))) as gpool, \
             tc.tile_pool(name="oh", bufs=int(os.environ.get("KBUFS", "4"))) as opool, \
             tc.tile_pool(name="xt", bufs=20) as xpool, \
             tc.tile_pool(name="ps", bufs=4, space="PSUM") as pspool, \
             tc.tile_pool(name="ps2", bufs=1, space="PSUM") as ps2pool, \
             tc.tile_pool(name="sm", bufs=1) as smpool, \
             tc.tile_pool(name="dram", bufs=1, space="DRAM") as dram:

            s_idxw = const.tile([128, nblk * BLK * 8], I16)
            s_srel = const.tile([128, nchp], F32)
            s_wnv = const.tile([128, nchp], F32)
            s_iota = const.tile([128, W], F32)
            s_bias = const.tile([128, C], F32)
            nc.sync.dma_start(s_idxw[:], t_idxw)
            nc.sync.dma_start(s_srel[:], t_srel)
            nc.sync.dma_start(s_wnv[:], t_wnv)
            nc.sync.dma_start(s_iota[:], t_iota)
            nc.sync.dma_start(s_bias[:], t_bias)

            for _rep in range(repeat):
              s_ST = const.tile([128, ntile, W], F32)
              s_fcp = const.tile([128, ntile, W], F32)
              nc.sync.dma_start(
                  s_fcp[:], t_fcp.rearrange("(t p) c -> p t c", p=128))
              s_d2 = const.tile([128, ntile], F32)
              nc.sync.dma_start(s_d2[:], t_d2)
              # init S^T with the self-loop diagonal term dinv^2 * fcT;
              # slot evacs then ADD into it (no post-loop barrier)
              nc.vector.tensor_tensor(
                  out=s_ST[:], in0=s_fcp[:],
                  in1=s_d2[:].unsqueeze(2).to_broadcast([128, ntile, W]),
                  op=mybir.AluOpType.mult)

              xt_tiles = []
              if phases >= 2:
                  # prefetch all x^T tiles now so phase 2 is pure-PE;
                  # these DMAs overlap the whole gather phase
                  for t in range(ntile):
                      xt = xpool.tile([128, B], F32, name="xt")
                      nc.sync.dma_start(
                          xt[:], t_xTp[t * 128:(t + 1) * 128, :])
                      xt_tiles.append(xt)

              g_tiles = [None] * nblk
              o_tiles = [None] * nblk
              emitted = [0]

              def ensure_block(b):
                  while emitted[0] <= b:
                      bb = emitted[0]
                      g_t = gpool.tile([128, BLK, ELEM], F32)
                      # 1024-idx single-packet gathers spread over 4 SWDGE
                      # queues: descriptor generation parallelizes across
                      # queues (HW-measured ~5x over one big gather).
                      for g in range(BLK // GB):
                          c0 = bb * BLK * 8 + g * GB * 8
                          i0 = (bb * BLK + g * GB) * P
                          valid = max(0, min(GB * P, nch_edges - i0))
                          nc.gpsimd.dma_gather(
                              out_ap=g_t[:, g * GB:(g + 1) * GB, :],
                              in_ap=t_table,
                              idxs_ap=s_idxw[:, c0:c0 + GB * 8],
                              num_idxs=GB * P,
                              num_idxs_reg=valid,
                              elem_size=ELEM,
                              single_packet=True,
                              queue_num=(bb * (BLK // GB) + g) % NQ,
                          )
                      o_t = opool.tile([128, BLK, W], F32)
                      nc.vector.tensor_tensor(
                          out=o_t[:],
                          in0=s_srel[:, bb * BLK:(bb + 1) * BLK]
                          .unsqueeze(2).to_broadcast([128, BLK, W]),
                          in1=s_iota[:].unsqueeze(1).to_broadcast([128, BLK, W]),
                          op=mybir.AluOpType.is_equal,
                      )
                      nc.vector.tensor_tensor(
                          out=o_t[:],
                          in0=o_t[:],
                          in1=s_wnv[:, bb * BLK:(bb + 1) * BLK]
                          .unsqueeze(2).to_broadcast([128, BLK, W]),
                          op=mybir.AluOpType.mult,
                      )
                      g_tiles[bb] = g_t
                      o_tiles[bb] = o_t
                      emitted[0] += 1

              # phase 1: per-slot accumulation of S^T tiles
              j = 0
              for s in range(nslot):
                  ps = pspool.tile([W, W], F32)
                  ks = K[s]
                  for t in range(ks):
                      b, jj = divmod(j, BLK)
                      ensure_block(b)
                      nc.tensor.matmul(
                          out=ps[:],
                          lhsT=o_tiles[b][:, jj, :],
                          rhs=g_tiles[b][:, jj, :W],
                          start=(t == 0),
                          stop=(t == ks - 1),
                      )
                      j += 1
                  po = W * (s % 4)
                  nc.vector.tensor_tensor(
                      out=s_ST[po:po + W, s // 4, :],
                      in0=s_ST[po:po + W, s // 4, :], in1=ps[:],
                      op=mybir.AluOpType.add)

              if phases >= 2:
                  # phase 2: partial logits = xTp.T @ S^T (contract over nodes)
                  pl = ps2pool.tile([B, W], F32)
                  for t in range(ntile):
                      nc.tensor.matmul(
                          out=pl[:],
                          lhsT=xt_tiles[t][:],
                          rhs=s_ST[:, t, :],
                          start=(t == 0),
                          stop=(t == ntile - 1),
                      )
                  s_plog = smpool.tile([B, W], F32)
                  nc.scalar.activation(s_plog[:], pl[:],
                                       mybir.ActivationFunctionType.Copy)

              if phases >= 3:
                  # phase 3: AllReduce partial logits across the 8 cores
                  b_in = dram.tile([B, W], F32)
                  b_out = dram.tile([B, W], F32)
                  nc.gpsimd.dma_start(b_in[:], s_plog[:])
                  nc.gpsimd.collective_compute(
                      "AllReduce", mybir.AluOpType.add,
                      replica_groups=[list(range(N_CORES))],
                      ins=[b_in.opt()], outs=[b_out.opt()],
                  )
                  s_log = smpool.tile([B, W], F32)
                  nc.sync.dma_start(s_log[:], b_out[:])

              if phases >= 4:
                  # phase 4: bias + softmax over the 30 real classes
                  s_logb = smpool.tile([B, C], F32)
                  nc.vector.tensor_tensor(out=s_logb[:], in0=s_log[:, :C],
                                          in1=s_bias[:],
                                          op=mybir.AluOpType.add)
                  s_negmax = smpool.tile([B, 1], F32)
                  nc.vector.tensor_reduce(out=s_negmax[:], in_=s_logb[:],
                                          axis=mybir.AxisListType.X,
                                          op=mybir.AluOpType.max, negate=True)
                  s_exp = smpool.tile([B, C], F32)
                  s_sum = smpool.tile([B, 1], F32)
                  nc.scalar.activation(s_exp[:], s_logb[:],
                                       mybir.ActivationFunctionType.Exp,
                                       bias=s_negmax[:, :1], scale=1.0,
                                       accum_out=s_sum[:])
                  s_rinv = smpool.tile([B, 1], F32)
                  nc.vector.reciprocal(s_rinv[:], s_sum[:])
                  s_probs = smpool.tile([B, C], F32)
                  nc.scalar.activation(s_probs[:], s_exp[:],
                                       mybir.ActivationFunctionType.Copy,
                                       scale=s_rinv[:, :1])
                  nc.sync.dma_start(t_out, s_probs[:])
              elif phases >= 2:
                  nc.sync.dma_start(t_out, s_plog[:, :C])
              else:
                  nc.sync.dma_start(t_out, s_ST[:B, 0, :C])

    nc.compile()
    _split_multi_waits(nc)
    return nc


# ---------------------------------------------------------------------------
def prepare(**inputs):
    """Preprocess + build + compile (cached). Returns (nc, in_maps)."""
    in_maps, meta = _preprocess(**inputs)
    key = (meta["nslot"], meta["K"], meta["nchp"], meta["ntile"])
    if key not in _CACHE:
        _CACHE[key] = _build(meta)
    return _CACHE[key], in_maps


def kernel(**inputs):
    nc, in_maps = prepare(**inputs)
    res = run_bass_kernel_spmd(nc, in_maps, core_ids=list(range(N_CORES)),
                               trace=False)
    return np.asarray(res.results[0]["out"], np.float32)



# revision 4
# speedup vs baseline: 1.0687x; 1.0687x over previous
"""Trainium2 Bass kernel for nn_DiffuseLR (SGConv K=1 diffusion + FC + softmax).

Math: out = softmax(x @ A^T @ fc_w^T + fc_b) with A the gcn-normalized
adjacency (incl. self loops). We exploit linearity: S^T = A^T @ fc_w^T is a
(N x C) "diffused classifier" computed by gather + one-hot matmul
(segment-sum over src), then logits = x @ S^T. The N-dim contraction is
sharded over 8 cores by src-node groups; per-core partial logits (128 x 30)
are AllReduced on-device and every core computes the final softmax.

Per-core device work:
  - dma_gather fc_w^T rows by edge dst (256B rows) for its edge shard
  - one-hot (src-relative, wn-weighted) matmuls accumulate S^T tiles in PSUM
  - FC phase: x^T tiles (host-permuted to this core's node order) @ S^T
  - AllReduce partial logits, add bias, softmax

Host side only: gcn normalization (per the sharding hint, the normalized
edge weight is replicated data), sorting/sharding of edges, and data layout
(transposes / padding / index packing).
"""
import os
import numpy as np

from concourse import bacc, bass, mybir, tile
from concourse.bass_utils import run_bass_kernel_spmd

N_CORES = 8
N = 20000          # nodes (features of x)
B = 128            # batch
C = 30             # classes
P = 128            # edges per matmul chunk (contraction dim)
W = 32             # src-node group width (onehot width)
BLK = 32           # chunks per gather/onehot block
ELEM = 64          # gathered row width in f32 (256 B, fc row padded 30->64)
GB = 8             # chunks per dma_gather instruction (1024 idx, 64
                   # descriptors per SDMA engine = single-packet limit)
NQ = 4             # SWDGE queues; queue q runs on Q7 cores 2q,2q+1 so 4
                   # queues parallelize descriptor generation 4x (measured)
G = N // W         # 625 src groups
F32 = mybir.dt.float32
I16 = mybir.dt.int16

_CACHE = {}


# ---------------------------------------------------------------------------
# workaround: this neuronxcc build accepts at most ONE sync wait per
# instruction; move extra waits onto same-engine NoOps placed just before.
def _split_multi_waits(nc):
    n_split = 0
    for func in nc.m.functions:
        for bb in func.blocks:
            insts = list(bb.instructions)
            if not any(
                i.sync_info is not None and len(i.sync_info.on_wait) > 1
                for i in insts
            ):
                continue
            new_list = []
            for inst in insts:
                si = inst.sync_info
                if si is not None and len(si.on_wait) > 1:
                    waits = list(si.on_wait)
                    for w in waits[:-1]:
                        nop = nc.engines[inst.engine].nop(
                            nofuse=True, hint="waitsplit"
                        )
                        cur = nc.cur_bb.bb
                        assert cur.instructions[-1].name == nop.ins.name
                        cur.instructions.pop()
                        nop.ins.sync_info = mybir.SyncInfo(
                            on_wait=[w], on_update=[]
                        )
                        new_list.append(nop.ins)
                        n_split += 1
                    si.on_wait = waits[-1:]
                new_list.append(inst)
            bb.instructions.clear()
            for i in new_list:
                bb.instructions.append(i)
    return n_split


# ---------------------------------------------------------------------------
def _preprocess(x, edge_index, edge_weight, fc_w, fc_b):
    """Host: normalize, sort by src, shard groups to cores, pack arrays."""
    src = np.asarray(edge_index[0], dtype=np.int64)
    dst = np.asarray(edge_index[1], dtype=np.int64)
    w = np.asarray(edge_weight, dtype=np.float64)
    loop = np.arange(N, dtype=np.int64)
    src_f = np.concatenate([src, loop])
    dst_f = np.concatenate([dst, loop])
    w_f = np.concatenate([w, np.ones(N)])

    deg = np.bincount(dst_f, weights=w_f, minlength=N)
    dinv = np.where(deg > 0, 1.0 / np.sqrt(np.maximum(deg, 1e-30)), 0.0)
    # self-loops are NOT gathered: their contribution is the diagonal term
    # S^T[n,:] += dinv[n]^2 * fcT[n,:], applied on-device after phase 1.
    wn = (dinv[src] * w * dinv[dst]).astype(np.float32)
    dinv2 = (dinv * dinv).astype(np.float32)

    order = np.argsort(src, kind="stable")
    src_s = src[order]
    dst_s = dst[order].astype(np.int16)
    wn_s = wn[order]

    gid = (src_s // W).astype(np.int64)
    cnt = np.bincount(gid, minlength=G)
    gstart = np.zeros(G + 1, np.int64)
    np.cumsum(cnt, out=gstart[1:])
    srel_all = (src_s - gid * W).astype(np.float32)

    chk = np.maximum((cnt + P - 1) // P, 1)
    meta_nch = None  # set below
    gorder = np.argsort(-cnt, kind="stable")
    nslot = (G + N_CORES - 1) // N_CORES               # 79
    K = np.array([chk[gorder[s * N_CORES]] for s in range(nslot)], np.int64)
    nch = int(K.sum())
    meta_nch = nch
    nchp = ((nch + BLK - 1) // BLK) * BLK
    nblk = nchp // BLK
    chunk_off = np.zeros(nslot + 1, np.int64)
    np.cumsum(K, out=chunk_off[1:])

    xT = np.ascontiguousarray(np.asarray(x, np.float32).T)  # (N, B)
    ntile = (nslot * W + 127) // 128                        # 20
    T = nchp * P

    table = np.zeros((N, ELEM), np.float32)
    table[:, :C] = np.asarray(fc_w, np.float32).T

    iota = np.broadcast_to(np.arange(W, dtype=np.float32), (128, W)).copy()
    bias = np.broadcast_to(np.asarray(fc_b, np.float32), (128, C)).copy()

    in_maps = []
    for c in range(N_CORES):
        gdst = np.zeros(T, np.int16)
        srel = np.full(T, 99.0, np.float32)
        wnv = np.zeros(T, np.float32)
        node_ids = np.zeros(ntile * 128, np.int64)
        valid_row = np.zeros(ntile * 128, bool)
        for s in range(nslot):
            r = s * N_CORES + c
            if r >= G:
                continue
            g = gorder[r]
            e0, n_e = gstart[g], cnt[g]
            base = chunk_off[s] * P
            gdst[base:base + n_e] = dst_s[e0:e0 + n_e]
            srel[base:base + n_e] = srel_all[e0:e0 + n_e]
            wnv[base:base + n_e] = wn_s[e0:e0 + n_e]
            node_ids[s * W:(s + 1) * W] = g * W + np.arange(W)
            valid_row[s * W:(s + 1) * W] = True
        # trailing dead chunks: -1 indices are trimmed by the gather ucode
        nch_edges = int(meta_nch) * P
        gdst[nch_edges:] = -1
        # idx wrapped per gather block: within block, flat k -> [k%16, k//16]
        idxw = np.concatenate(
            [gdst[b * BLK * P:(b + 1) * BLK * P].reshape(-1, 16).T
             for b in range(nblk)], axis=1)           # (16, nblk*BLK*8)
        idxw = np.tile(idxw, (8, 1))                  # (128, ...)
        in_maps.append({
            "table": table,
            "idxw": np.ascontiguousarray(idxw),
            "srel": np.ascontiguousarray(srel.reshape(nchp, P).T),
            "wnv": np.ascontiguousarray(wnv.reshape(nchp, P).T),
            "xTp": np.ascontiguousarray(xT[node_ids, :]),
            "fcp": np.ascontiguousarray(table[node_ids, :W]),
            "d2": np.ascontiguousarray(
                (dinv2[node_ids] * valid_row).astype(np.float32)
                .reshape(ntile, 128).T),
            "iota": iota,
            "bias": bias,
        })
    meta = {"nslot": nslot, "K": tuple(int(k) for k in K),
            "nchp": nchp, "nblk": nblk, "ntile": ntile, "nch": nch}
    return in_maps, meta


# ---------------------------------------------------------------------------
def _build(meta, phases=4, max_slots=None, repeat=1):
    nslot, K, nchp, nblk, ntile = (meta["nslot"], meta["K"], meta["nchp"],
                                   meta["nblk"], meta["ntile"])
    nch_edges = meta["nch"] * P
    if max_slots is not None:
        nslot = min(nslot, max_slots)
    nc = bacc.Bacc("TRN2", debug=False, enable_asserts=False,
                   num_devices=N_CORES, num_swdge_queues=NQ)
    t_table = nc.dram_tensor("table", [N, ELEM], F32, kind="ExternalInput").ap()
    t_idxw = nc.dram_tensor("idxw", [128, nblk * BLK * 8], I16,
                            kind="ExternalInput").ap()
    t_srel = nc.dram_tensor("srel", [128, nchp], F32, kind="ExternalInput").ap()
    t_wnv = nc.dram_tensor("wnv", [128, nchp], F32, kind="ExternalInput").ap()
    t_xTp = nc.dram_tensor("xTp", [ntile * 128, B], F32,
                           kind="ExternalInput").ap()
    t_fcp = nc.dram_tensor("fcp", [ntile * 128, W], F32,
                           kind="ExternalInput").ap()
    t_d2 = nc.dram_tensor("d2", [128, ntile], F32, kind="ExternalInput").ap()
    t_iota = nc.dram_tensor("iota", [128, W], F32, kind="ExternalInput").ap()
    t_bias = nc.dram_tensor("bias", [128, C], F32, kind="ExternalInput").ap()
    t_out = nc.dram_tensor("out", [B, C], F32, kind="ExternalOutput").ap()

    with tile.TileContext(nc) as tc:
        with tc.tile_pool(name="const", bufs=1) as const, \
             tc.tile_pool(name="gath", bufs=int(os.environ.get("KBUFS", "4"))) as gpool, \
             tc.tile_pool(name="oh", bufs=int(os.environ.get("KBUFS", "4"))) as opool, \
             tc.tile_pool(name="xt", bufs=40) as xpool, \
             tc.tile_pool(name="stb", bufs=2) as stpool, \
             tc.tile_pool(name="ps", bufs=4, space="PSUM") as pspool, \
             tc.tile_pool(name="ps2", bufs=1, space="PSUM") as ps2pool, \
             tc.tile_pool(name="sm", bufs=1) as smpool, \
             tc.tile_pool(name="dram", bufs=1, space="DRAM") as dram:

            s_idxw = const.tile([128, nblk * BLK * 8], I16)
            s_srel = const.tile([128, nchp], F32)
            s_wnv = const.tile([128, nchp], F32)
            s_iota = const.tile([128, W], F32)
            s_bias = const.tile([128, C], F32)
            nc.sync.dma_start(s_idxw[:], t_idxw)
            nc.sync.dma_start(s_srel[:], t_srel)
            nc.sync.dma_start(s_wnv[:], t_wnv)
            nc.sync.dma_start(s_iota[:], t_iota)
            nc.sync.dma_start(s_bias[:], t_bias)

            for _rep in range(repeat):
              # rep-scoped tiles live in a bufs=2 pool so iteration r+1's
              # init/prefetch does not serialize on iteration r's final FC
              # reads (pipelines the repeat bodies).
              s_ST = stpool.tile([128, ntile, W], F32)
              s_fcp = stpool.tile([128, ntile, W], F32)
              nc.sync.dma_start(
                  s_fcp[:], t_fcp.rearrange("(t p) c -> p t c", p=128))
              s_d2 = stpool.tile([128, ntile], F32)
              nc.sync.dma_start(s_d2[:], t_d2)
              # init S^T with the self-loop diagonal term dinv^2 * fcT;
              # slot evacs then ADD into it (no post-loop barrier)
              nc.vector.tensor_tensor(
                  out=s_ST[:], in0=s_fcp[:],
                  in1=s_d2[:].unsqueeze(2).to_broadcast([128, ntile, W]),
                  op=mybir.AluOpType.mult)

              xt_tiles = []
              if phases >= 2:
                  # prefetch all x^T tiles now so phase 2 is pure-PE;
                  # these DMAs overlap the whole gather phase
                  for t in range(ntile):
                      xt = xpool.tile([128, B], F32, name="xt")
                      nc.sync.dma_start(
                          xt[:], t_xTp[t * 128:(t + 1) * 128, :])
                      xt_tiles.append(xt)

              g_tiles = [None] * nblk
              o_tiles = [None] * nblk
              emitted = [0]

              def ensure_block(b):
                  while emitted[0] <= b:
                      bb = emitted[0]
                      g_t = gpool.tile([128, BLK, ELEM], F32)
                      # 1024-idx single-packet gathers spread over 4 SWDGE
                      # queues: descriptor generation parallelizes across
                      # queues (HW-measured ~5x over one big gather).
                      for g in range(BLK // GB):
                          c0 = bb * BLK * 8 + g * GB * 8
                          i0 = (bb * BLK + g * GB) * P
                          valid = max(0, min(GB * P, nch_edges - i0))
                          nc.gpsimd.dma_gather(
                              out_ap=g_t[:, g * GB:(g + 1) * GB, :],
                              in_ap=t_table,
                              idxs_ap=s_idxw[:, c0:c0 + GB * 8],
                              num_idxs=GB * P,
                              num_idxs_reg=valid,
                              elem_size=ELEM,
                              single_packet=True,
                              queue_num=(bb * (BLK // GB) + g) % NQ,
                          )
                      o_t = opool.tile([128, BLK, W], F32)
                      nc.vector.tensor_tensor(
                          out=o_t[:],
                          in0=s_srel[:, bb * BLK:(bb + 1) * BLK]
                          .unsqueeze(2).to_broadcast([128, BLK, W]),
                          in1=s_iota[:].unsqueeze(1).to_broadcast([128, BLK, W]),
                          op=mybir.AluOpType.is_equal,
                      )
                      nc.vector.tensor_tensor(
                          out=o_t[:],
                          in0=o_t[:],
                          in1=s_wnv[:, bb * BLK:(bb + 1) * BLK]
                          .unsqueeze(2).to_broadcast([128, BLK, W]),
                          op=mybir.AluOpType.mult,
                      )
                      g_tiles[bb] = g_t
                      o_tiles[bb] = o_t
                      emitted[0] += 1

              # phase 1: per-slot accumulation of S^T tiles
              j = 0
              for s in range(nslot):
                  ps = pspool.tile([W, W], F32)
                  ks = K[s]
                  for t in range(ks):
                      b, jj = divmod(j, BLK)
                      ensure_block(b)
                      nc.tensor.matmul(
                          out=ps[:],
                          lhsT=o_tiles[b][:, jj, :],
                          rhs=g_tiles[b][:, jj, :W],
                          start=(t == 0),
                          stop=(t == ks - 1),
                      )
                      j += 1
                  po = W * (s % 4)
                  nc.vector.tensor_tensor(
                      out=s_ST[po:po + W, s // 4, :],
                      in0=s_ST[po:po + W, s // 4, :], in1=ps[:],
                      op=mybir.AluOpType.add)

              if phases >= 2:
                  # phase 2: partial logits = xTp.T @ S^T (contract over nodes)
                  pl = ps2pool.tile([B, W], F32)
                  for t in range(ntile):
                      nc.tensor.matmul(
                          out=pl[:],
                          lhsT=xt_tiles[t][:],
                          rhs=s_ST[:, t, :],
                          start=(t == 0),
                          stop=(t == ntile - 1),
                      )
                  s_plog = smpool.tile([B, W], F32)
                  nc.scalar.activation(s_plog[:], pl[:],
                                       mybir.ActivationFunctionType.Copy)

              if phases >= 3:
                  # phase 3: AllReduce partial logits across the 8 cores
                  b_in = dram.tile([B, W], F32)
                  b_out = dram.tile([B, W], F32)
                  nc.gpsimd.dma_start(b_in[:], s_plog[:])
                  nc.gpsimd.collective_compute(
                      "AllReduce", mybir.AluOpType.add,
                      replica_groups=[list(range(N_CORES))],
                      ins=[b_in.opt()], outs=[b_out.opt()],
                  )
                  s_log = smpool.tile([B, W], F32)
                  nc.sync.dma_start(s_log[:], b_out[:])

              if phases >= 4:
                  # phase 4: bias + softmax over the 30 real classes
                  s_logb = smpool.tile([B, C], F32)
                  nc.vector.tensor_tensor(out=s_logb[:], in0=s_log[:, :C],
                                          in1=s_bias[:],
                                          op=mybir.AluOpType.add)
                  s_negmax = smpool.tile([B, 1], F32)
                  nc.vector.tensor_reduce(out=s_negmax[:], in_=s_logb[:],
                                          axis=mybir.AxisListType.X,
                                          op=mybir.AluOpType.max, negate=True)
                  s_exp = smpool.tile([B, C], F32)
                  s_sum = smpool.tile([B, 1], F32)
                  nc.scalar.activation(s_exp[:], s_logb[:],
                                       mybir.ActivationFunctionType.Exp,
                                       bias=s_negmax[:, :1], scale=1.0,
                                       accum_out=s_sum[:])
                  s_rinv = smpool.tile([B, 1], F32)
                  nc.vector.reciprocal(s_rinv[:], s_sum[:])
                  s_probs = smpool.tile([B, C], F32)
                  nc.scalar.activation(s_probs[:], s_exp[:],
                                       mybir.ActivationFunctionType.Copy,
                                       scale=s_rinv[:, :1])
                  nc.sync.dma_start(t_out, s_probs[:])
              elif phases >= 2:
                  nc.sync.dma_start(t_out, s_plog[:, :C])
              else:
                  nc.sync.dma_start(t_out, s_ST[:B, 0, :C])

    nc.compile()
    _split_multi_waits(nc)
    return nc


# ---------------------------------------------------------------------------
def prepare(**inputs):
    """Preprocess + build + compile (cached). Returns (nc, in_maps)."""
    in_maps, meta = _preprocess(**inputs)
    key = (meta["nslot"], meta["K"], meta["nchp"], meta["ntile"])
    if key not in _CACHE:
        _CACHE[key] = _build(meta)
    return _CACHE[key], in_maps


def kernel(**inputs):
    nc, in_maps = prepare(**inputs)
    res = run_bass_kernel_spmd(nc, in_maps, core_ids=list(range(N_CORES)),
                               trace=False)
    return np.asarray(res.results[0]["out"], np.float32)

